# revision 37
# baseline (speedup 1.0000x reference)
"""MiniRocket feature extraction kernel for Trainium2 (8 NeuronCores, data parallel).

Contract: kernel(**inputs) takes the FULL inputs (as produced by setup_inputs())
and returns the FULL [64, 1344] float32 output. Internally the batch dim is
sharded 8-ways across the 8 NeuronCores; all other tensors are small replicated
constants that are preprocessed on the host into matmul weights / bias tables.

Math (per batch b, dilation d, kernel k, feature f):
    resp[k, l] = sum_{c,j} mask[d,k,c] * kern[k,j] * x[b, l + (j-4)*dil, c]
    feat[k, f] = w[k] * #{l in W_k : resp[k,l] > bias[d,k,f]}
    out        = (feat - mean) / std
where W_k is the full [0,L) window (even parity of d_idx+k, w=1/L) or the
interior [p, L-p) (odd parity, p = 4*dil, w=1/(L-2p)).

Device mapping (v4 -- 128-row packed PSUM, PSUM-direct counting, edge trick):
  - The per-(b,d) responses are packed 336-rows-per-batch into 24 logical
    PSUM tiles [128, 2048] (3 per batch) via quadrant-legal matmul sub-blocks
    (out partition offsets in {0,32,64,96}; <=32-row blocks anywhere, <=64-row
    blocks at {0,64}).  Counting cost is per-COLUMN, so 128-row tiles cut the
    count-op count from 128 to 96 vs the unpacked [84, *] layout.
  - The interior-window (odd-parity) trick is folded into the matmul: each
    patch carries a 73rd row holding the edge-indicator e_d[l] (1 on the
    2*pad edge columns), and the weight matrix gives that row -1e4 for
    odd-parity kernels.  Edge columns of odd rows come out of the matmul
    already poisoned below every bias -- no separate poison ops.
  - PPV counting reads resp DIRECTLY FROM PSUM, one op per (tile, feature):
      * DVE: tensor_scalar(is_gt, add, accum_out) -> direct count,
      * ACT: Sign(resp - b) with accum -> count = S/2 + L/2,
    with the (tile, feature) -> engine assignment chosen so both engines
    carry equal time (ACT ops are ~10% cheaper than DVE ops).
  - Final affine (count*A + B) folds the PPV weight, mean and std; A/B/bias
    tables are host-built per (tile-row, tile-feature-col) so dead partition
    rows (the 48 quadrant-packing crumbs per batch) are simply zeroed.
  - Patch tiles [73, 2048] rotate through 8 slots; slot s always serves
    dilation s%4, so row 72 (the e_d row) is written once upfront and the
    per-(b,d) patch DMA only rewrites rows 0..71 straight from DRAM.

walrus in this toolchain encodes at most ONE sync wait per compute/DMA
instruction; _legalize_sync_waits rewrites Tile's emitted waits to fit: a
transitive-closure (vector-clock) min-cover prunes redundant waits, extra
Matmult waits are hoisted onto the preceding Ldweights, and DMA waits park on
earlier free PE slots.  CRITICAL semantics baked into the pruner: an engine's
OWN semaphore tick is completion-level knowledge only and must never propagate
through the engine's instruction stream -- accumulator-drain aux ops (and
posted writes) lag the next instruction's dispatch on this silicon.
"""

import os
import sys

for _p in (
    "/root/.axon_site",
    "/root/.axon_site/_ro/trn_rl_repo",
    "/root/.axon_site/_ro/pypackages",
    "/opt/trn_rl_repo",
):
    if os.path.isdir(_p) and _p not in sys.path:
        sys.path.append(_p)

import numpy as np

B, L, C = 64, 2048, 8
DILATIONS = (1, 2, 4, 8)
D = 4
K = 84
F = 4
KERNEL_LEN = 9
NCORES = 8
BPC = B // NCORES  # batches per core
PAD = 32  # max shift = 4 * max(dil)
LP = L + 2 * PAD  # padded length
CE = C + 4  # channels + 4 edge-indicator pseudo-channels (one per dilation)
PR = CE * KERNEL_LEN  # patch rows (108): (c, j) pairs incl. edge rows
NT = (BPC * D * K) // 128  # 21 perfectly-packed [128, 2048] tiles per core
EDGE_W = -30000.0  # edge-poison weight on the edge rows (odd-parity kernels)

# Perfect packing: global row g = b*336 + d*84 + k maps to tile g//128,
# partition g%128 -- 2688 rows fill 21 tiles exactly.  Every matmul writes
# the FULL 128 partitions at offset 0 (always quadrant-legal) with zero
# weight columns outside its (b, d)-block's partition range; overlapping
# pieces within a tile compose via PSUM accumulation (start=False).


def _pieces():
    """List of (b, d, tile, row_a, row_b, k0): (b,d)-block parts per tile."""
    out = []
    for b in range(BPC):
        for d in range(D):
            g0 = b * (D * K) + d * K
            g1 = g0 + K
            g = g0
            while g < g1:
                t = g // 128
                ge = min(g1, (t + 1) * 128)
                out.append((b, d, t, g - 128 * t, ge - 128 * t, K - (g1 - g)))
                g = ge
    return sorted(out, key=lambda p: (p[2], p[0], p[1], p[3]))

_PROGRAM_CACHE: dict = {}


def _row_map():
    """(tile, partition) -> (b, d, k)."""
    m = [[None] * 128 for _ in range(NT)]
    for t in range(NT):
        for p in range(128):
            g = 128 * t + p
            b = g // (D * K)
            r = g % (D * K)
            m[t][p] = (b, r // K, r % K)
    return m


# Engine split: each [128, 1024] HALF-tile is read by exactly ONE engine
# (Tile serializes same-allocation reads from two engines -- a RAR artifact
# that would ping-pong DVE and ACT).  Half 0 of every tile -> DVE (is_gt
# counts), half 1 -> ACT (Sign counts); the two partial counts fold in the
# final affine: count = cnt0 + S1/2 + 512.


def _host_constants(kernels, channel_masks, bias_matrices, feature_mean, feature_std):
    """Build wT [73, D*K] f16 and cst [128, NT*4 * 4] f32 (bias, A_dve, A_act, B)."""
    kernels = np.asarray(kernels, np.float32)
    channel_masks = np.asarray(channel_masks, np.float32)
    bias_matrices = np.asarray(bias_matrices, np.float32)
    feature_mean = np.asarray(feature_mean, np.float32).reshape(D, K, F)
    feature_std = np.asarray(feature_std, np.float32).reshape(D, K, F)

    # weights: one [PR, 128] column-block per PIECE; within the block,
    # column p holds W[(c,j), k0+p-row_a] for p in [row_a, row_b) and zero
    # outside, so the matmul's full-128-partition write is a no-op on other
    # blocks' rows (they compose via PSUM accumulation).  Row 72+9*d (tap
    # j=0 of dilation d's edge pseudo-channel) = EDGE_W for odd-parity
    # kernels (their edge columns must count as "below every bias").
    pieces = _pieces()
    wfull = np.zeros((D, PR, K), np.float32)
    for d_idx in range(D):
        w = channel_masks[d_idx][:, :, None] * kernels[:, None, :]  # [K, C, 9]
        wfull[d_idx, 0:72, :] = w.reshape(K, C * KERNEL_LEN).T
        parity_odd = (d_idx + np.arange(K)) % 2 == 1
        wfull[d_idx, 72 + 9 * d_idx, :] = np.where(parity_odd, EDGE_W, 0.0)
    wT = np.zeros((PR, len(pieces) * 128), np.float16)
    for j, (b, d_idx, t, ra, rb, k0) in enumerate(pieces):
        wT[:, j * 128 + ra : j * 128 + rb] = wfull[d_idx][:, k0 : k0 + (rb - ra)].astype(
            np.float16
        )

    rows = _row_map()
    ncol = NT * F
    bias_d = np.zeros((128, ncol), np.float32)
    bias_a = np.zeros((128, ncol), np.float32)
    a_t = np.zeros((128, ncol), np.float32)
    a2_t = np.zeros((128, ncol), np.float32)
    b2_t = np.zeros((128, ncol), np.float32)
    for t in range(NT):
        for p in range(128):
            _b, d_idx, k = rows[t][p]
            pad = 4 * DILATIONS[d_idx]
            odd = (d_idx + k) % 2 == 1
            w_sel = 1.0 / (L - 2 * pad) if odd else 1.0 / L
            for f in range(F):
                i = t * F + f
                bb = bias_matrices[d_idx, k, f]
                mm = feature_mean[d_idx, k, f]
                ss = feature_std[d_idx, k, f]
                # count = cnt0 (DVE is_gt over half 0)
                #       + S1/2 + 512 (ACT Sign over half 1; edge poison
                #         contributes -1 like a below-bias sample)
                # out = (w*count - m)/s = cnt0*A + S1*A2 + B2
                bias_d[p, i] = bb
                bias_a[p, i] = -bb  # ACT bias is ADDED: Sign(resp + (-b))
                a_t[p, i] = w_sel / ss
                a2_t[p, i] = w_sel / (2.0 * ss)
                b2_t[p, i] = (w_sel * 512.0 - mm) / ss
    cst = np.concatenate([bias_d, bias_a, a_t, a2_t, b2_t], axis=1)
    return wT, cst


def _build_program():
    """Build the Bass/Tile program (same NEFF for all 8 cores)."""
    from contextlib import ExitStack

    import bass_rust
    import concourse.bass as bass
    import concourse.tile as tile
    from concourse import mybir

    f16 = mybir.dt.float16
    f32 = mybir.dt.float32
    A = mybir.AluOpType
    ACT = mybir.ActivationFunctionType

    ncol = NT * F

    nc = bass.Bass()
    xT = nc.declare_dram_parameter("xT", [BPC * CE, LP], f16, isOutput=False)
    wT = nc.declare_dram_parameter("wT", [PR, len(_pieces()) * 128], f16, isOutput=False)
    cst = nc.declare_dram_parameter("cst", [128, 5 * ncol], f32, isOutput=False)
    out = nc.declare_dram_parameter("out", [128, ncol], f32, isOutput=True)

    def patch_src(b, dil):
        """DRAM view: 9 dilation-shifted [CE, L] windows of batch b, c-major
        (includes the 4 edge pseudo-channels as rows 72..107)."""
        c = xT.ap().copy()
        c.offset = b * CE * LP + PAD - 4 * dil
        c.ap = bass_rust.VecI64Pair([[LP, CE], [dil, KERNEL_LEN], [1, L]])
        return c

    NSLOT = 32  # one slot per (b, d): no slot reuse, no WAR waits

    with tile.TileContext(nc) as tc, ExitStack() as ctx:
        cpool = ctx.enter_context(tc.tile_pool(name="const", bufs=1))
        patch_pool = ctx.enter_context(tc.tile_pool(name="patch", bufs=1))
        psum_pool = ctx.enter_context(tc.tile_pool(name="psum", bufs=4, space="PSUM"))
        tr_pool = ctx.enter_context(tc.tile_pool(name="tr", bufs=8))
        tra_pool = ctx.enter_context(tc.tile_pool(name="tra", bufs=8))
        cnt_pool = ctx.enter_context(tc.tile_pool(name="cnt", bufs=1))
        osb_pool = ctx.enter_context(tc.tile_pool(name="osb", bufs=1))

        npieces = len(_pieces())
        nfirst = sum(1 for p in _pieces() if p[2] <= 2)
        wsb_a = cpool.tile([108, nfirst * 128], f16)
        wsb_b = cpool.tile([108, (npieces - nfirst) * 128], f16)
        wsrc = wT.ap()
        nc.sync.dma_start(wsb_a[:], wsrc[:, 0 : nfirst * 128])
        csb = cpool.tile([128, 5 * ncol], f32)

        # patch slots: patch bodies stream through the (otherwise idle)
        # GpSimd SWDGE queues IN PARALLEL with SP issuing the constants --
        # DMA issue costs ~600ns of engine time each, so splitting the
        # issue work across two engines shortens the lead-in.
        patches = [
            patch_pool.tile([108, L], f16, name=f"patch{s}") for s in range(NSLOT)
        ]

        def issue_patch(b, d_idx):
            s = (b * D + d_idx) % NSLOT
            nc.sync.dma_start(patches[s][:], patch_src(b, DILATIONS[d_idx]))

        for b in range(2):
            for d_idx in range(D):
                issue_patch(b, d_idx)
        nc.sync.dma_start(wsb_b[:], wsrc[:, nfirst * 128 :])
        nc.sync.dma_start(csb[:], cst.ap())
        for b in range(2, BPC):
            for d_idx in range(D):
                issue_patch(b, d_idx)

        cnt_d = cnt_pool.tile([128, ncol], f32)
        cnt_a = cnt_pool.tile([128, ncol], f32)
        scr_d = cnt_pool.tile([128, 1], f32)
        scr_a = cnt_pool.tile([128, 1], f32)
        osb = osb_pool.tile([128, ncol], f32)

        # Touch csb once from DVE and ACT so its DMA-completion tick is in
        # both engines' vector clocks; later ops then carry at most one wait.
        nc.vector.tensor_copy(scr_d[:], csb[:, 0:1])
        nc.scalar.activation(scr_a[:], csb[0:128, 0:1], ACT.Copy)

        pieces = _pieces()
        by_tile = {}
        for j, pc in enumerate(pieces):
            by_tile.setdefault(pc[2], []).append((j, pc))
        # last piece index consuming each batch's patches (for prefetch)
        last_piece_of_batch = {}
        for j, (b, d_idx, t, ra, rb, k0) in enumerate(pieces):
            last_piece_of_batch[b] = max(last_piece_of_batch.get(b, -1), j)
        mm_boundaries = []
        mm_count = 0
        for t in range(NT):
            tp = by_tile[t]
            for h in range(2):
                ps = psum_pool.tile([128, 1024], f32, name="ps")
                for pi, (j, (b, d_idx, tt, ra, rb, k0)) in enumerate(tp):
                    patch = patches[(b * D + d_idx) % NSLOT]
                    for ch in (2 * h, 2 * h + 1):
                        cc = (ch - 2 * h) * 512
                        wtile, wj = (
                            (wsb_a, j) if j < nfirst else (wsb_b, j - nfirst)
                        )
                        nc.tensor.matmul(
                            ps[:, cc : cc + 512],
                            lhsT=wtile[:, wj * 128 : (wj + 1) * 128],
                            rhs=patch[:, ch * 512 : (ch + 1) * 512],
                            start=(pi == 0),
                            stop=(pi == len(tp) - 1),
                        )
                        mm_count += 1
                mm_boundaries.append(mm_count)
                for f in range(F):
                    i = t * F + f
                    if h == 0:
                        trash = tr_pool.tile([128, 1024], f16, name="trash")
                        nc.vector.tensor_scalar(
                            trash[:],
                            ps[:],
                            csb[:, i : i + 1],
                            None,
                            A.is_gt,
                            A.add,
                            accum_out=cnt_d[:, i : i + 1],
                        )
                    else:
                        trash_a = tra_pool.tile([128, 1024], f16, name="trash_a")
                        nc.scalar.activation(
                            trash_a[:],
                            ps[:],
                            ACT.Sign,
                            bias=csb[:, ncol + i : ncol + i + 1],
                            accum_out=cnt_a[:, i : i + 1],
                        )
        nc._mm_boundaries = set(mm_boundaries)

        # fold + affine: osb = cnt0*A + S1*A2 + B2  (count = cnt0 + S1/2
        # + 512 folded into the tables; host unscrambles columns)
        nc.vector.tensor_tensor(
            cnt_d[:], cnt_d[:], csb[:, 2 * ncol : 3 * ncol], A.mult
        )
        nc.vector.tensor_tensor(
            cnt_a[:], cnt_a[:], csb[:, 3 * ncol : 4 * ncol], A.mult
        )
        nc.vector.tensor_tensor(osb[:], cnt_d[:], cnt_a[:], A.add)
        nc.vector.tensor_tensor(
            osb[:], osb[:], csb[:, 4 * ncol : 5 * ncol], A.add
        )

        nc.sync.dma_start(out.ap(), osb[:])

    _legalize_sync_waits(nc, bass_rust)
    return nc


def _legalize_sync_waits(nc, bass_rust):
    """walrus encodes at most ONE sync wait per compute/DMA instruction.
    Rewrites, validated in the CoreSim race detector and on hardware:
     1. Transitive-closure (vector-clock) min-cover prunes redundant waits.
     2. Extra Matmult waits hoist onto the immediately-preceding Ldweights.
     3. Remaining multi-waits on DMAs park on earlier free PE slots.
     4. Kernel-tail SP drain waits prune to (at most) the output-store queue.
    """
    blocks = list(nc.m.functions[0].blocks)
    end_blk = next(b for b in blocks if b.name.endswith("_end"))

    max_waited: dict = {}
    for blk in blocks:
        if blk is end_blk:
            continue
        for inst in blk.instructions:
            si = inst.sync_info
            for w in si.on_wait if si and si.on_wait else []:
                if w.wait_value > max_waited.get(w.ant_name, -1):
                    max_waited[w.ant_name] = w.wait_value

    body = [b for b in blocks if b is not end_blk and not b.name == "main"]
    know_after: dict = {}  # stream knowledge (excludes own sem: accum aux lag)
    know_full: dict = {}  # completion knowledge (includes own sem updates)
    producers: dict = {}  # sem -> list of (value, inst_idx, is_dma)
    prev_on_engine: dict = {}
    eng_stream: dict = {}  # engine -> its instructions in program order
    insts = [i for b in body for i in b.instructions]

    def covered(know, sem, val):
        return know.get(sem, -1) >= val

    for idx, inst in enumerate(insts):
        eng = str(inst.engine).split(".")[-1]
        si = inst.sync_info
        is_dma = inst.opcode == "DMACopy"
        know = dict(know_after.get(prev_on_engine.get(eng), {}))
        waits = list(si.on_wait) if si and si.on_wait else []
        if waits:
            # knowledge each wait would contribute
            contrib = []
            for w in waits:
                c = {}
                for v, pidx, pdma in producers.get(w.ant_name, []):
                    if v >= w.wait_value:
                        c = dict(know_full.get(pidx, {}))
                        break
                c[w.ant_name] = max(c.get(w.ant_name, -1), w.wait_value)
                contrib.append(c)
            # smallest subset of waits whose merged transitive knowledge
            # (plus same-engine knowledge) covers every wait
            from itertools import combinations

            need = [
                i
                for i, w in enumerate(waits)
                if not covered(know, w.ant_name, w.wait_value)
            ]
            best = None
            for sz in range(0, len(need) + 1):
                for sub in combinations(need, sz):
                    merged = dict(know)
                    for i in sub:
                        for s, v in contrib[i].items():
                            if merged.get(s, -1) < v:
                                merged[s] = v
                    if all(
                        covered(merged, waits[i].ant_name, waits[i].wait_value)
                        for i in need
                    ):
                        best = (sub, merged)
                        break
                if best is not None:
                    break
            assert best is not None
            know = best[1]
            waits = [waits[i] for i in best[0]]
        if len(waits) > 1:
            # Hoist extra waits onto earlier wait-free instructions of the
            # SAME engine (engines execute in order, so a wait satisfied
            # before an earlier instruction is satisfied before this one).
            # DMA-queue waits may park anywhere AFTER the producing enqueue
            # (the enqueue does not depend on this engine, so no cycle);
            # engine-sem waits keep a tight 8-instruction window, inside
            # which wait producers depend only on work preceding the window.
            eng_insts = eng_stream.get(eng, [])
            waits.sort(key=lambda w: not w.ant_name.startswith("DMA"))
            kept_w = []
            while len(kept_w) + len(waits) > 1 and waits:
                w = waits.pop(0)
                if w.ant_name.startswith("DMA"):
                    plist = producers.get(w.ant_name, [])
                    pidx = -1
                    for v, pi, pdma in plist:
                        if v >= w.wait_value:
                            pidx = pi
                            break
                    lo = 0
                    while lo < len(eng_insts) and eng_insts[lo][0] <= pidx:
                        lo += 1
                    lo = max(lo, 0)
                else:
                    lo = max(0, len(eng_insts) - 8)
                placed = False
                for j in range(len(eng_insts) - 1, lo - 1, -1):
                    cand = eng_insts[j][1]
                    csi = cand.sync_info
                    if csi is not None and csi.on_wait:
                        continue
                    if csi is None:
                        csi = bass_rust.SyncInfo(on_wait=[], on_update=[])
                        cand.sync_info = csi
                    csi.on_wait = [w]
                    placed = True
                    break
                if not placed:
                    if os.environ.get("LEGALIZE_DEBUG"):
                        print(
                            f"DEBUG place-fail {inst.name} w={w.ant_name}>={w.wait_value} "
                            f"lo={lo} n_eng={len(eng_insts)} "
                            f"tail_busy={[(g, i.opcode, bool(i.sync_info and i.sync_info.on_wait)) for g, i in eng_insts[max(0,lo):][-12:]]}"
                        )
                    kept_w.append(w)
            waits = kept_w + waits
        assert len(waits) <= 1, (
            f"{inst.name} {inst.opcode} still has waits "
            f"{[(w.ant_name, w.wait_value) for w in waits]}"
        )
        if si is not None:
            si.on_wait = waits
        elif waits:
            inst.sync_info = bass_rust.SyncInfo(on_wait=waits, on_update=[])
        # record updates (update_value is an INCREMENT; waits are cumulative
        # thresholds, so track running totals per semaphore). An instruction
        # with an accumulator output drains it via a lagging aux op: its sem
        # tick is completion-level knowledge only and must NOT propagate
        # through the engine stream (the next instruction may start first).
        # DMA enqueues complete asynchronously.
        full = dict(know)
        if si and si.on_update:
            for u in si.on_update:
                plist = producers.setdefault(u.ant_name, [])
                total = (plist[-1][0] if plist else 0) + u.update_value
                plist.append((total, idx, is_dma))
                if not is_dma:
                    if full.get(u.ant_name, -1) < total:
                        full[u.ant_name] = total
        know_after[idx] = know
        know_full[idx] = full
        prev_on_engine[eng] = idx
        eng_stream.setdefault(eng, []).append((idx, inst))

    _thin_pe_stream(nc, insts, blocks, end_blk)

    # (4) tail drain
    end_insts = list(end_blk.instructions)
    tail = end_insts[0]
    assert tail.opcode == "Drain", f"unexpected end block head {tail.opcode}"
    si = tail.sync_info
    if si and len(si.on_wait) > 1:
        eng_pfx = ("Activation_", "PE_", "DVE_", "Pool_", "SP_")
        keep = [
            w
            for w in si.on_wait
            if not w.ant_name.startswith(eng_pfx)
            and max_waited.get(w.ant_name, -1) < w.wait_value
        ]
        if len(keep) > 1:
            # spill extras onto zero-wait drains before the sem reset
            spill_slots = []
            for inst in end_insts[1:]:
                if inst.opcode == "ISA":
                    break
                isi = inst.sync_info
                if inst.opcode == "Drain" and (not isi or not isi.on_wait):
                    spill_slots.append(inst)
            assert len(spill_slots) >= len(keep) - 1, (
                f"tail drain needs {len(keep)} wait slots, "
                f"only {1 + len(spill_slots)} available"
            )
            for w, slot in zip(keep[1:], spill_slots):
                ssi = slot.sync_info
                if ssi is None:
                    ssi = bass_rust.SyncInfo(on_wait=[], on_update=[])
                    slot.sync_info = ssi
                ssi.on_wait = [w]
            keep = keep[:1]
        si.on_wait = keep


def _thin_pe_stream(nc, insts, blocks, end_blk):
    """Cut PE-stream overhead, per the tensor-engine guidance that
    per-matmul semaphore increments serialize (~26ns each) and break
    back-to-back matmul pipelining:
      1. Keep the PE completion-sem update only on each PSUM tile's LAST
         matmul (matmuls complete in pc order, so the last tick implies the
         rest); remap every wait on that sem accordingly (rounding a wait up
         to its tile's last matmul is always sound -- it waits longer).
      2. Drop a Ldweights when the previous surviving PE instruction chain
         loads the SAME weights AP (consecutive matmuls reuse the loaded
         weights); only parameter-free ones (no waits, no updates) go.
    """
    boundaries = getattr(nc, "_mm_boundaries", set())
    # find the PE completion sem: the one Matmults update
    pe_sem = None
    for inst in insts:
        if inst.opcode == "Matmult" and inst.sync_info and inst.sync_info.on_update:
            pe_sem = inst.sync_info.on_update[0].ant_name
            break
    if pe_sem is None:
        return
    # pass 1: strip non-tile-last matmul updates, build old->new total map
    old_total = 0
    kept_totals = []  # (old_total_reached, new_total)
    new_total = 0
    for inst in insts:
        si = inst.sync_info
        ups = list(si.on_update) if si and si.on_update else []
        for u in ups:
            if u.ant_name != pe_sem:
                continue
            old_total += u.update_value
            if inst.opcode == "Matmult" and old_total not in boundaries:
                si.on_update = [x for x in si.on_update if x is not u]
            else:
                new_total += u.update_value
                kept_totals.append((old_total, new_total))

    def remap(v):
        for old, new in kept_totals:
            if old >= v:
                return new
        return kept_totals[-1][1] if kept_totals else v

    for blk in blocks:
        for inst in blk.instructions:
            si = inst.sync_info
            if not si or not si.on_wait:
                continue
            changed = False
            ws = []
            for w in si.on_wait:
                if w.ant_name == pe_sem:
                    nv = remap(w.wait_value)
                    if nv != w.wait_value:
                        w.wait_value = nv
                        changed = True
                ws.append(w)
            if changed:
                si.on_wait = ws

    # pass 2: dedup consecutive identical Ldweights on the PE stream
    pe_insts = [
        i for i in insts if str(i.engine).split(".")[-1] == "PE"
    ]
    last_w = None
    to_remove = set()
    for inst in pe_insts:
        if inst.opcode == "Ldweights":
            si = inst.sync_info
            key = str(inst.ins[0])
            if (
                key == last_w
                and not (si and (si.on_wait or si.on_update))
            ):
                to_remove.add(id(inst))
            last_w = key
        elif inst.opcode != "Matmult":
            last_w = None  # barriers etc. may clobber the PE array state
    if to_remove:
        for blk in blocks:
            kept = [i for i in blk.instructions if id(i) not in to_remove]
            if len(kept) != len(list(blk.instructions)):
                blk.instructions = kept


def _get_program():
    if "nc" not in _PROGRAM_CACHE:
        _PROGRAM_CACHE["nc"] = _build_program()
    return _PROGRAM_CACHE["nc"]


def _prep_x(x):
    """[64, 2048, 8] f32 -> per-core [BPC*12, 2112] f16 slices.

    Channels 0..7 are the (padded, channel-major) data; channels 8..11 hold
    the per-dilation edge-indicator patterns U_d with U_d[PAD-4*dil+l] =
    e_d[l], so the patch AP picks up each dilation's e-row as tap j=0 of its
    fake channel -- one DMA covers data AND edge rows.
    """
    xt = np.ascontiguousarray(np.asarray(x, np.float32).transpose(0, 2, 1))
    xp = np.zeros((B, CE, LP), np.float16)
    xp[:, :C, PAD : PAD + L] = xt.astype(np.float16)
    for d_idx, dil in enumerate(DILATIONS):
        pad = 4 * dil
        e = np.zeros(L, np.float16)
        e[:pad] = 1.0
        e[L - pad :] = 1.0
        xp[:, C + d_idx, PAD - pad : PAD - pad + L] = e
    return [
        xp[i * BPC : (i + 1) * BPC].reshape(BPC * CE, LP) for i in range(NCORES)
    ]


def kernel(
    x,
    kernels,
    channel_masks,
    bias_matrices,
    feature_mean,
    feature_std,
    _trace=False,
    _sim=False,
):
    wT, cst = _host_constants(
        kernels, channel_masks, bias_matrices, feature_mean, feature_std
    )
    x_slices = _prep_x(x)
    nc = _get_program()

    in_maps = [
        {"xT": x_slices[i], "wT": wT, "cst": cst}
        for i in range(NCORES)
    ]

    if _sim:
        import concourse.bass_interp as bass_interp

        try:
            nc.detect_race_conditions = False
        except Exception:
            pass
        sim = bass_interp.MultiCoreSim(nc, 1)
        sim.cores[0].assign_tensors(in_maps[0])
        sim.simulate()
        dev_outs = [np.array(sim.cores[0].tensor("out"))]
        full = np.zeros((B, 1344), np.float32)
        _scatter(full[:BPC], dev_outs[0])
        _PROGRAM_CACHE["exec_time_ns"] = None
        return full

    if _trace:
        _install_ntff_hook_shim()

    from concourse.bass_utils import run_bass_kernel_spmd

    res = run_bass_kernel_spmd(
        nc,
        in_maps,
        core_ids=list(range(NCORES)),
        trace=_trace,
        trace_cores=list(range(NCORES)) if _trace else None,
    )
    _PROGRAM_CACHE["exec_time_ns"] = res.exec_time_ns
    _PROGRAM_CACHE["mean_exec_time_ns"] = res.mean_exec_time_ns
    _PROGRAM_CACHE["trace"] = res.instructions_and_trace

    full = np.empty((B, 1344), np.float32)
    for i in range(NCORES):
        _scatter(full[i * BPC : (i + 1) * BPC], res.results[i]["out"])
    return full


def _install_ntff_hook_shim():
    """The image's antenv lacks axon_hooks; provide it so run_bass_kernel_spmd
    trace=True can capture NTFF profiles through the axon tunnel."""
    import sys as _sys
    import types

    try:
        from antenv.axon_hooks import get_axon_ntff_profile_hook  # noqa: F401

        return
    except ImportError:
        pass
    from trn_agent_boot.trn_boot import _ntff_profile_via_ctypes

    hook = _ntff_profile_via_ctypes("/opt/axon/libaxon_pjrt.so")
    mod = types.ModuleType("antenv.axon_hooks")
    mod.get_axon_ntff_profile_hook = lambda: hook
    mod.set_axon_ntff_profile_hook = lambda h: None
    _sys.modules["antenv.axon_hooks"] = mod


def _scatter_index():
    """Precompute (col, partition) -> flat output index maps per device col."""
    if "scatter" in _PROGRAM_CACHE:
        return _PROGRAM_CACHE["scatter"]
    rows = _row_map()
    all_cols = [(t, f) for t in range(NT) for f in range(F)]
    ncol = len(all_cols)
    b_of = np.zeros((ncol, 128), np.int64)
    feat_of = np.zeros((ncol, 128), np.int64)
    for c, (t, f) in enumerate(all_cols):
        for p in range(128):
            b, d_idx, k = rows[t][p]
            b_of[c, p] = b
            feat_of[c, p] = d_idx * K * F + k * F + f
    _PROGRAM_CACHE["scatter"] = (b_of, feat_of)
    return b_of, feat_of


def _scatter(dst, dev_out):
    """dev_out [128, NT*4] -> dst [BPC, 1344] in reference order."""
    dev = np.asarray(dev_out, np.float32)
    b_of, feat_of = _scatter_index()
    for c in range(feat_of.shape[0]):
        dst[b_of[:, :][c], feat_of[c]] = dev[:, c]


# revision 38
# speedup vs baseline: 1.1837x; 1.1837x over previous
"""MiniRocket feature extraction kernel for Trainium2 (8 NeuronCores, data parallel).

Contract: kernel(**inputs) takes the FULL inputs (as produced by setup_inputs())
and returns the FULL [64, 1344] float32 output. Internally the batch dim is
sharded 8-ways across the 8 NeuronCores; all other tensors are small replicated
constants that are preprocessed on the host into matmul weights / bias tables.

Math (per batch b, dilation d, kernel k, feature f):
    resp[k, l] = sum_{c,j} mask[d,k,c] * kern[k,j] * x[b, l + (j-4)*dil, c]
    feat[k, f] = w[k] * #{l in W_k : resp[k,l] > bias[d,k,f]}
    out        = (feat - mean) / std
where W_k is the full [0,L) window (even parity of d_idx+k, w=1/L) or the
interior [p, L-p) (odd parity, p = 4*dil, w=1/(L-2p)).

Device mapping (v4 -- 128-row packed PSUM, PSUM-direct counting, edge trick):
  - The per-(b,d) responses are packed 336-rows-per-batch into 24 logical
    PSUM tiles [128, 2048] (3 per batch) via quadrant-legal matmul sub-blocks
    (out partition offsets in {0,32,64,96}; <=32-row blocks anywhere, <=64-row
    blocks at {0,64}).  Counting cost is per-COLUMN, so 128-row tiles cut the
    count-op count from 128 to 96 vs the unpacked [84, *] layout.
  - The interior-window (odd-parity) trick is folded into the matmul: each
    patch carries a 73rd row holding the edge-indicator e_d[l] (1 on the
    2*pad edge columns), and the weight matrix gives that row -1e4 for
    odd-parity kernels.  Edge columns of odd rows come out of the matmul
    already poisoned below every bias -- no separate poison ops.
  - PPV counting reads resp DIRECTLY FROM PSUM, one op per (tile, feature):
      * DVE: tensor_scalar(is_gt, add, accum_out) -> direct count,
      * ACT: Sign(resp - b) with accum -> count = S/2 + L/2,
    with the (tile, feature) -> engine assignment chosen so both engines
    carry equal time (ACT ops are ~10% cheaper than DVE ops).
  - Final affine (count*A + B) folds the PPV weight, mean and std; A/B/bias
    tables are host-built per (tile-row, tile-feature-col) so dead partition
    rows (the 48 quadrant-packing crumbs per batch) are simply zeroed.
  - Patch tiles [73, 2048] rotate through 8 slots; slot s always serves
    dilation s%4, so row 72 (the e_d row) is written once upfront and the
    per-(b,d) patch DMA only rewrites rows 0..71 straight from DRAM.

walrus in this toolchain encodes at most ONE sync wait per compute/DMA
instruction; _legalize_sync_waits rewrites Tile's emitted waits to fit: a
transitive-closure (vector-clock) min-cover prunes redundant waits, extra
Matmult waits are hoisted onto the preceding Ldweights, and DMA waits park on
earlier free PE slots.  CRITICAL semantics baked into the pruner: an engine's
OWN semaphore tick is completion-level knowledge only and must never propagate
through the engine's instruction stream -- accumulator-drain aux ops (and
posted writes) lag the next instruction's dispatch on this silicon.
"""

import os
import sys

for _p in (
    "/root/.axon_site",
    "/root/.axon_site/_ro/trn_rl_repo",
    "/root/.axon_site/_ro/pypackages",
    "/opt/trn_rl_repo",
):
    if os.path.isdir(_p) and _p not in sys.path:
        sys.path.append(_p)

import numpy as np

B, L, C = 64, 2048, 8
DILATIONS = (1, 2, 4, 8)
D = 4
K = 84
F = 4
KERNEL_LEN = 9
NCORES = 8
BPC = B // NCORES  # batches per core
PAD = 32  # max shift = 4 * max(dil)
LP = L + 2 * PAD  # padded length
CE = C + 4  # channels + 4 edge-indicator pseudo-channels (one per dilation)
PR = CE * KERNEL_LEN  # patch rows (108): (c, j) pairs incl. edge rows
NT = (BPC * D * K) // 128  # 21 perfectly-packed [128, 2048] tiles per core
EDGE_W = -30000.0  # edge-poison weight on the edge rows (odd-parity kernels)

# Perfect packing: global row g = b*336 + d*84 + k maps to tile g//128,
# partition g%128 -- 2688 rows fill 21 tiles exactly.  Every matmul writes
# the FULL 128 partitions at offset 0 (always quadrant-legal) with zero
# weight columns outside its (b, d)-block's partition range; overlapping
# pieces within a tile compose via PSUM accumulation (start=False).


def _pieces():
    """List of (b, d, tile, row_a, row_b, k0): (b,d)-block parts per tile."""
    out = []
    for b in range(BPC):
        for d in range(D):
            g0 = b * (D * K) + d * K
            g1 = g0 + K
            g = g0
            while g < g1:
                t = g // 128
                ge = min(g1, (t + 1) * 128)
                out.append((b, d, t, g - 128 * t, ge - 128 * t, K - (g1 - g)))
                g = ge
    return sorted(out, key=lambda p: (p[2], p[0], p[1], p[3]))

_PROGRAM_CACHE: dict = {}


def _row_map():
    """(tile, partition) -> (b, d, k)."""
    m = [[None] * 128 for _ in range(NT)]
    for t in range(NT):
        for p in range(128):
            g = 128 * t + p
            b = g // (D * K)
            r = g % (D * K)
            m[t][p] = (b, r // K, r % K)
    return m


# Engine split: each [128, 1024] HALF-tile is read by exactly ONE engine
# (Tile serializes same-allocation reads from two engines -- a RAR artifact
# that would ping-pong DVE and ACT).  Half 0 of every tile -> DVE (is_gt
# counts), half 1 -> ACT (Sign counts); the two partial counts fold in the
# final affine: count = cnt0 + S1/2 + 512.


def _host_constants(kernels, channel_masks, bias_matrices, feature_mean, feature_std):
    """Build wT [73, D*K] f16 and cst [128, NT*4 * 4] f32 (bias, A_dve, A_act, B)."""
    kernels = np.asarray(kernels, np.float32)
    channel_masks = np.asarray(channel_masks, np.float32)
    bias_matrices = np.asarray(bias_matrices, np.float32)
    feature_mean = np.asarray(feature_mean, np.float32).reshape(D, K, F)
    feature_std = np.asarray(feature_std, np.float32).reshape(D, K, F)

    # weights: one [PR, 128] column-block per PIECE; within the block,
    # column p holds W[(c,j), k0+p-row_a] for p in [row_a, row_b) and zero
    # outside, so the matmul's full-128-partition write is a no-op on other
    # blocks' rows (they compose via PSUM accumulation).  Row 72+9*d (tap
    # j=0 of dilation d's edge pseudo-channel) = EDGE_W for odd-parity
    # kernels (their edge columns must count as "below every bias").
    pieces = _pieces()
    wfull = np.zeros((D, PR, K), np.float32)
    for d_idx in range(D):
        w = channel_masks[d_idx][:, :, None] * kernels[:, None, :]  # [K, C, 9]
        wfull[d_idx, 0:72, :] = w.reshape(K, C * KERNEL_LEN).T
        parity_odd = (d_idx + np.arange(K)) % 2 == 1
        wfull[d_idx, 72 + 9 * d_idx, :] = np.where(parity_odd, EDGE_W, 0.0)
    wT = np.zeros((PR, len(pieces) * 128), np.float16)
    for j, (b, d_idx, t, ra, rb, k0) in enumerate(pieces):
        wT[:, j * 128 + ra : j * 128 + rb] = wfull[d_idx][:, k0 : k0 + (rb - ra)].astype(
            np.float16
        )

    rows = _row_map()
    ncol = NT * F
    bias_d = np.zeros((128, ncol), np.float32)
    bias_a = np.zeros((128, ncol), np.float32)
    a_t = np.zeros((128, ncol), np.float32)
    a2_t = np.zeros((128, ncol), np.float32)
    b2_t = np.zeros((128, ncol), np.float32)
    for t in range(NT):
        for p in range(128):
            _b, d_idx, k = rows[t][p]
            pad = 4 * DILATIONS[d_idx]
            odd = (d_idx + k) % 2 == 1
            w_sel = 1.0 / (L - 2 * pad) if odd else 1.0 / L
            for f in range(F):
                i = t * F + f
                bb = bias_matrices[d_idx, k, f]
                mm = feature_mean[d_idx, k, f]
                ss = feature_std[d_idx, k, f]
                # count = cnt0 (DVE is_gt over half 0)
                #       + S1/2 + 512 (ACT Sign over half 1; edge poison
                #         contributes -1 like a below-bias sample)
                # out = (w*count - m)/s = cnt0*A + S1*A2 + B2
                bias_d[p, i] = bb
                bias_a[p, i] = -bb  # ACT bias is ADDED: Sign(resp + (-b))
                a_t[p, i] = w_sel / ss
                a2_t[p, i] = w_sel / (2.0 * ss)
                b2_t[p, i] = (w_sel * 512.0 - mm) / ss
    cst = np.concatenate([bias_d, bias_a, a_t, a2_t, b2_t], axis=1)
    return wT, cst


def _build_program():
    """Build the Bass/Tile program (same NEFF for all 8 cores)."""
    from contextlib import ExitStack

    import bass_rust
    import concourse.bass as bass
    import concourse.tile as tile
    from concourse import mybir

    f16 = mybir.dt.float16
    f32 = mybir.dt.float32
    A = mybir.AluOpType
    ACT = mybir.ActivationFunctionType

    ncol = NT * F

    nc = bass.Bass()
    xT = nc.declare_dram_parameter("xT", [BPC * CE, LP], f16, isOutput=False)
    wT = nc.declare_dram_parameter("wT", [PR, len(_pieces()) * 128], f16, isOutput=False)
    cst = nc.declare_dram_parameter("cst", [128, 5 * ncol], f32, isOutput=False)
    out = nc.declare_dram_parameter("out", [128, ncol], f32, isOutput=True)

    def patch_src(b, dil):
        """DRAM view: 9 dilation-shifted [CE, L] windows of batch b, c-major
        (includes the 4 edge pseudo-channels as rows 72..107)."""
        c = xT.ap().copy()
        c.offset = b * CE * LP + PAD - 4 * dil
        c.ap = bass_rust.VecI64Pair([[LP, CE], [dil, KERNEL_LEN], [1, L]])
        return c

    NSLOT = 32  # one slot per (b, d): no slot reuse, no WAR waits

    with tile.TileContext(nc) as tc, ExitStack() as ctx:
        cpool = ctx.enter_context(tc.tile_pool(name="const", bufs=1))
        patch_pool = ctx.enter_context(tc.tile_pool(name="patch", bufs=1))
        psum_pool = ctx.enter_context(tc.tile_pool(name="psum", bufs=4, space="PSUM"))
        tr_pool = ctx.enter_context(tc.tile_pool(name="tr", bufs=8))
        tra_pool = ctx.enter_context(tc.tile_pool(name="tra", bufs=8))
        cnt_pool = ctx.enter_context(tc.tile_pool(name="cnt", bufs=1))
        osb_pool = ctx.enter_context(tc.tile_pool(name="osb", bufs=1))

        npieces = len(_pieces())
        nfirst = sum(1 for p in _pieces() if p[2] <= 2)
        wsb_a = cpool.tile([108, nfirst * 128], f16)
        wsb_b = cpool.tile([108, (npieces - nfirst) * 128], f16)
        wsrc = wT.ap()
        nc.sync.dma_start(wsb_a[:], wsrc[:, 0 : nfirst * 128])
        csb = cpool.tile([128, 5 * ncol], f32)

        # patch slots: patch bodies stream through the (otherwise idle)
        # GpSimd SWDGE queues IN PARALLEL with SP issuing the constants --
        # DMA issue costs ~600ns of engine time each, so splitting the
        # issue work across two engines shortens the lead-in.
        patches = [
            patch_pool.tile([108, L], f16, name=f"patch{s}") for s in range(NSLOT)
        ]

        def issue_patch(b, d_idx):
            s = (b * D + d_idx) % NSLOT
            nc.sync.dma_start(patches[s][:], patch_src(b, DILATIONS[d_idx]))

        for b in range(2):
            for d_idx in range(D):
                issue_patch(b, d_idx)
        nc.sync.dma_start(wsb_b[:], wsrc[:, nfirst * 128 :])
        nc.sync.dma_start(csb[:], cst.ap())
        for d_idx in range(D):
            issue_patch(2, d_idx)

        cnt_d = cnt_pool.tile([128, ncol], f32)
        cnt_a = cnt_pool.tile([128, ncol], f32)
        scr_d = cnt_pool.tile([128, 1], f32)
        scr_a = cnt_pool.tile([128, 1], f32)
        osb = osb_pool.tile([128, ncol], f32)

        # Touch csb once from DVE and ACT so its DMA-completion tick is in
        # both engines' vector clocks; later ops then carry at most one wait.
        nc.vector.tensor_copy(scr_d[:], csb[:, 0:1])
        nc.scalar.activation(scr_a[:], csb[0:128, 0:1], ACT.Copy)

        pieces = _pieces()
        by_tile = {}
        for j, pc in enumerate(pieces):
            by_tile.setdefault(pc[2], []).append((j, pc))
        # last piece index consuming each batch's patches (for prefetch)
        last_piece_of_batch = {}
        for j, (b, d_idx, t, ra, rb, k0) in enumerate(pieces):
            last_piece_of_batch[b] = max(last_piece_of_batch.get(b, -1), j)
        mm_boundaries = []
        mm_count = 0
        for t in range(NT):
            tp = by_tile[t]
            for h in range(2):
                ps = psum_pool.tile([128, 1024], f32, name="ps")
                for pi, (j, (b, d_idx, tt, ra, rb, k0)) in enumerate(tp):
                    patch = patches[(b * D + d_idx) % NSLOT]
                    for ch in (2 * h, 2 * h + 1):
                        cc = (ch - 2 * h) * 512
                        wtile, wj = (
                            (wsb_a, j) if j < nfirst else (wsb_b, j - nfirst)
                        )
                        nc.tensor.matmul(
                            ps[:, cc : cc + 512],
                            lhsT=wtile[:, wj * 128 : (wj + 1) * 128],
                            rhs=patch[:, ch * 512 : (ch + 1) * 512],
                            start=(pi == 0),
                            stop=(pi == len(tp) - 1),
                        )
                        mm_count += 1
                mm_boundaries.append(mm_count)
                # prefetch 3 batches out once batch b's patches are done
                # with (fresh slots: no WAR wait, just the queue-ring wait)
                if h == 1:
                    for bb in range(BPC - 3):
                        if last_piece_of_batch[bb] in [j for j, _ in tp]:
                            for d_idx in range(D):
                                issue_patch(bb + 3, d_idx)
                for f in range(F):
                    i = t * F + f
                    if h == 0:
                        trash = tr_pool.tile([128, 1024], f16, name="trash")
                        nc.vector.tensor_scalar(
                            trash[:],
                            ps[:],
                            csb[:, i : i + 1],
                            None,
                            A.is_gt,
                            A.add,
                            accum_out=cnt_d[:, i : i + 1],
                        )
                    else:
                        trash_a = tra_pool.tile([128, 1024], f16, name="trash_a")
                        nc.scalar.activation(
                            trash_a[:],
                            ps[:],
                            ACT.Sign,
                            bias=csb[:, ncol + i : ncol + i + 1],
                            accum_out=cnt_a[:, i : i + 1],
                        )
        nc._mm_boundaries = set(mm_boundaries)

        # fold + affine: osb = cnt0*A + S1*A2 + B2  (count = cnt0 + S1/2
        # + 512 folded into the tables; host unscrambles columns)
        nc.vector.tensor_tensor(
            cnt_d[:], cnt_d[:], csb[:, 2 * ncol : 3 * ncol], A.mult
        )
        nc.vector.tensor_tensor(
            cnt_a[:], cnt_a[:], csb[:, 3 * ncol : 4 * ncol], A.mult
        )
        nc.vector.tensor_tensor(osb[:], cnt_d[:], cnt_a[:], A.add)
        nc.vector.tensor_tensor(
            osb[:], osb[:], csb[:, 4 * ncol : 5 * ncol], A.add
        )

        nc.sync.dma_start(out.ap(), osb[:])

    _legalize_sync_waits(nc, bass_rust)
    return nc


def _legalize_sync_waits(nc, bass_rust):
    """walrus encodes at most ONE sync wait per compute/DMA instruction.
    Rewrites, validated in the CoreSim race detector and on hardware:
     1. Transitive-closure (vector-clock) min-cover prunes redundant waits.
     2. Extra Matmult waits hoist onto the immediately-preceding Ldweights.
     3. Remaining multi-waits on DMAs park on earlier free PE slots.
     4. Kernel-tail SP drain waits prune to (at most) the output-store queue.
    """
    blocks = list(nc.m.functions[0].blocks)
    end_blk = next(b for b in blocks if b.name.endswith("_end"))

    max_waited: dict = {}
    for blk in blocks:
        if blk is end_blk:
            continue
        for inst in blk.instructions:
            si = inst.sync_info
            for w in si.on_wait if si and si.on_wait else []:
                if w.wait_value > max_waited.get(w.ant_name, -1):
                    max_waited[w.ant_name] = w.wait_value

    body = [b for b in blocks if b is not end_blk and not b.name == "main"]
    know_after: dict = {}  # stream knowledge (excludes own sem: accum aux lag)
    know_full: dict = {}  # completion knowledge (includes own sem updates)
    producers: dict = {}  # sem -> list of (value, inst_idx, is_dma)
    prev_on_engine: dict = {}
    eng_stream: dict = {}  # engine -> its instructions in program order
    insts = [i for b in body for i in b.instructions]

    def covered(know, sem, val):
        return know.get(sem, -1) >= val

    for idx, inst in enumerate(insts):
        eng = str(inst.engine).split(".")[-1]
        si = inst.sync_info
        is_dma = inst.opcode == "DMACopy"
        know = dict(know_after.get(prev_on_engine.get(eng), {}))
        waits = list(si.on_wait) if si and si.on_wait else []
        if waits:
            # knowledge each wait would contribute
            contrib = []
            for w in waits:
                c = {}
                for v, pidx, pdma in producers.get(w.ant_name, []):
                    if v >= w.wait_value:
                        c = dict(know_full.get(pidx, {}))
                        break
                c[w.ant_name] = max(c.get(w.ant_name, -1), w.wait_value)
                contrib.append(c)
            # smallest subset of waits whose merged transitive knowledge
            # (plus same-engine knowledge) covers every wait
            from itertools import combinations

            need = [
                i
                for i, w in enumerate(waits)
                if not covered(know, w.ant_name, w.wait_value)
            ]
            best = None
            for sz in range(0, len(need) + 1):
                for sub in combinations(need, sz):
                    merged = dict(know)
                    for i in sub:
                        for s, v in contrib[i].items():
                            if merged.get(s, -1) < v:
                                merged[s] = v
                    if all(
                        covered(merged, waits[i].ant_name, waits[i].wait_value)
                        for i in need
                    ):
                        best = (sub, merged)
                        break
                if best is not None:
                    break
            assert best is not None
            know = best[1]
            waits = [waits[i] for i in best[0]]
        if len(waits) > 1:
            # Hoist extra waits onto earlier wait-free instructions of the
            # SAME engine (engines execute in order, so a wait satisfied
            # before an earlier instruction is satisfied before this one).
            # DMA-queue waits may park anywhere AFTER the producing enqueue
            # (the enqueue does not depend on this engine, so no cycle);
            # engine-sem waits keep a tight 8-instruction window, inside
            # which wait producers depend only on work preceding the window.
            eng_insts = eng_stream.get(eng, [])
            waits.sort(key=lambda w: not w.ant_name.startswith("DMA"))
            kept_w = []
            while len(kept_w) + len(waits) > 1 and waits:
                w = waits.pop(0)
                if w.ant_name.startswith("DMA"):
                    plist = producers.get(w.ant_name, [])
                    pidx = -1
                    for v, pi, pdma in plist:
                        if v >= w.wait_value:
                            pidx = pi
                            break
                    lo = 0
                    while lo < len(eng_insts) and eng_insts[lo][0] <= pidx:
                        lo += 1
                    lo = max(lo, 0)
                else:
                    lo = max(0, len(eng_insts) - 8)
                placed = False
                for j in range(len(eng_insts) - 1, lo - 1, -1):
                    cand = eng_insts[j][1]
                    csi = cand.sync_info
                    if csi is not None and csi.on_wait:
                        continue
                    if csi is None:
                        csi = bass_rust.SyncInfo(on_wait=[], on_update=[])
                        cand.sync_info = csi
                    csi.on_wait = [w]
                    placed = True
                    break
                if not placed:
                    if os.environ.get("LEGALIZE_DEBUG"):
                        print(
                            f"DEBUG place-fail {inst.name} w={w.ant_name}>={w.wait_value} "
                            f"lo={lo} n_eng={len(eng_insts)} "
                            f"tail_busy={[(g, i.opcode, bool(i.sync_info and i.sync_info.on_wait)) for g, i in eng_insts[max(0,lo):][-12:]]}"
                        )
                    kept_w.append(w)
            waits = kept_w + waits
        assert len(waits) <= 1, (
            f"{inst.name} {inst.opcode} still has waits "
            f"{[(w.ant_name, w.wait_value) for w in waits]}"
        )
        if si is not None:
            si.on_wait = waits
        elif waits:
            inst.sync_info = bass_rust.SyncInfo(on_wait=waits, on_update=[])
        # record updates (update_value is an INCREMENT; waits are cumulative
        # thresholds, so track running totals per semaphore). An instruction
        # with an accumulator output drains it via a lagging aux op: its sem
        # tick is completion-level knowledge only and must NOT propagate
        # through the engine stream (the next instruction may start first).
        # DMA enqueues complete asynchronously.
        full = dict(know)
        if si and si.on_update:
            for u in si.on_update:
                plist = producers.setdefault(u.ant_name, [])
                total = (plist[-1][0] if plist else 0) + u.update_value
                plist.append((total, idx, is_dma))
                if not is_dma:
                    if full.get(u.ant_name, -1) < total:
                        full[u.ant_name] = total
        know_after[idx] = know
        know_full[idx] = full
        prev_on_engine[eng] = idx
        eng_stream.setdefault(eng, []).append((idx, inst))

    _thin_pe_stream(nc, insts, blocks, end_blk)

    # (4) tail drain
    end_insts = list(end_blk.instructions)
    tail = end_insts[0]
    assert tail.opcode == "Drain", f"unexpected end block head {tail.opcode}"
    si = tail.sync_info
    if si and len(si.on_wait) > 1:
        eng_pfx = ("Activation_", "PE_", "DVE_", "Pool_", "SP_")
        keep = [
            w
            for w in si.on_wait
            if not w.ant_name.startswith(eng_pfx)
            and max_waited.get(w.ant_name, -1) < w.wait_value
        ]
        if len(keep) > 1:
            # spill extras onto zero-wait drains before the sem reset
            spill_slots = []
            for inst in end_insts[1:]:
                if inst.opcode == "ISA":
                    break
                isi = inst.sync_info
                if inst.opcode == "Drain" and (not isi or not isi.on_wait):
                    spill_slots.append(inst)
            assert len(spill_slots) >= len(keep) - 1, (
                f"tail drain needs {len(keep)} wait slots, "
                f"only {1 + len(spill_slots)} available"
            )
            for w, slot in zip(keep[1:], spill_slots):
                ssi = slot.sync_info
                if ssi is None:
                    ssi = bass_rust.SyncInfo(on_wait=[], on_update=[])
                    slot.sync_info = ssi
                ssi.on_wait = [w]
            keep = keep[:1]
        si.on_wait = keep


def _thin_pe_stream(nc, insts, blocks, end_blk):
    """Cut PE-stream overhead, per the tensor-engine guidance that
    per-matmul semaphore increments serialize (~26ns each) and break
    back-to-back matmul pipelining:
      1. Keep the PE completion-sem update only on each PSUM tile's LAST
         matmul (matmuls complete in pc order, so the last tick implies the
         rest); remap every wait on that sem accordingly (rounding a wait up
         to its tile's last matmul is always sound -- it waits longer).
      2. Drop a Ldweights when the previous surviving PE instruction chain
         loads the SAME weights AP (consecutive matmuls reuse the loaded
         weights); only parameter-free ones (no waits, no updates) go.
    """
    boundaries = getattr(nc, "_mm_boundaries", set())
    # find the PE completion sem: the one Matmults update
    pe_sem = None
    for inst in insts:
        if inst.opcode == "Matmult" and inst.sync_info and inst.sync_info.on_update:
            pe_sem = inst.sync_info.on_update[0].ant_name
            break
    if pe_sem is None:
        return
    # pass 1: strip non-tile-last matmul updates, build old->new total map
    old_total = 0
    kept_totals = []  # (old_total_reached, new_total)
    new_total = 0
    for inst in insts:
        si = inst.sync_info
        ups = list(si.on_update) if si and si.on_update else []
        for u in ups:
            if u.ant_name != pe_sem:
                continue
            old_total += u.update_value
            if inst.opcode == "Matmult" and old_total not in boundaries:
                si.on_update = [x for x in si.on_update if x is not u]
            else:
                new_total += u.update_value
                kept_totals.append((old_total, new_total))

    def remap(v):
        for old, new in kept_totals:
            if old >= v:
                return new
        return kept_totals[-1][1] if kept_totals else v

    for blk in blocks:
        for inst in blk.instructions:
            si = inst.sync_info
            if not si or not si.on_wait:
                continue
            changed = False
            ws = []
            for w in si.on_wait:
                if w.ant_name == pe_sem:
                    nv = remap(w.wait_value)
                    if nv != w.wait_value:
                        w.wait_value = nv
                        changed = True
                ws.append(w)
            if changed:
                si.on_wait = ws

    # pass 2: dedup consecutive identical Ldweights on the PE stream
    pe_insts = [
        i for i in insts if str(i.engine).split(".")[-1] == "PE"
    ]
    last_w = None
    to_remove = set()
    for inst in pe_insts:
        if inst.opcode == "Ldweights":
            si = inst.sync_info
            key = str(inst.ins[0])
            if (
                key == last_w
                and not (si and (si.on_wait or si.on_update))
            ):
                to_remove.add(id(inst))
            last_w = key
        elif inst.opcode != "Matmult":
            last_w = None  # barriers etc. may clobber the PE array state
    if to_remove:
        for blk in blocks:
            kept = [i for i in blk.instructions if id(i) not in to_remove]
            if len(kept) != len(list(blk.instructions)):
                blk.instructions = kept


def _get_program():
    if "nc" not in _PROGRAM_CACHE:
        _PROGRAM_CACHE["nc"] = _build_program()
    return _PROGRAM_CACHE["nc"]


def _prep_x(x):
    """[64, 2048, 8] f32 -> per-core [BPC*12, 2112] f16 slices.

    Channels 0..7 are the (padded, channel-major) data; channels 8..11 hold
    the per-dilation edge-indicator patterns U_d with U_d[PAD-4*dil+l] =
    e_d[l], so the patch AP picks up each dilation's e-row as tap j=0 of its
    fake channel -- one DMA covers data AND edge rows.
    """
    xt = np.ascontiguousarray(np.asarray(x, np.float32).transpose(0, 2, 1))
    xp = np.zeros((B, CE, LP), np.float16)
    xp[:, :C, PAD : PAD + L] = xt.astype(np.float16)
    for d_idx, dil in enumerate(DILATIONS):
        pad = 4 * dil
        e = np.zeros(L, np.float16)
        e[:pad] = 1.0
        e[L - pad :] = 1.0
        xp[:, C + d_idx, PAD - pad : PAD - pad + L] = e
    return [
        xp[i * BPC : (i + 1) * BPC].reshape(BPC * CE, LP) for i in range(NCORES)
    ]


def kernel(
    x,
    kernels,
    channel_masks,
    bias_matrices,
    feature_mean,
    feature_std,
    _trace=False,
    _sim=False,
):
    wT, cst = _host_constants(
        kernels, channel_masks, bias_matrices, feature_mean, feature_std
    )
    x_slices = _prep_x(x)
    nc = _get_program()

    in_maps = [
        {"xT": x_slices[i], "wT": wT, "cst": cst}
        for i in range(NCORES)
    ]

    if _sim:
        import concourse.bass_interp as bass_interp

        try:
            nc.detect_race_conditions = False
        except Exception:
            pass
        sim = bass_interp.MultiCoreSim(nc, 1)
        sim.cores[0].assign_tensors(in_maps[0])
        sim.simulate()
        dev_outs = [np.array(sim.cores[0].tensor("out"))]
        full = np.zeros((B, 1344), np.float32)
        _scatter(full[:BPC], dev_outs[0])
        _PROGRAM_CACHE["exec_time_ns"] = None
        return full

    if _trace:
        _install_ntff_hook_shim()

    from concourse.bass_utils import run_bass_kernel_spmd

    res = run_bass_kernel_spmd(
        nc,
        in_maps,
        core_ids=list(range(NCORES)),
        trace=_trace,
        trace_cores=list(range(NCORES)) if _trace else None,
    )
    _PROGRAM_CACHE["exec_time_ns"] = res.exec_time_ns
    _PROGRAM_CACHE["mean_exec_time_ns"] = res.mean_exec_time_ns
    _PROGRAM_CACHE["trace"] = res.instructions_and_trace

    full = np.empty((B, 1344), np.float32)
    for i in range(NCORES):
        _scatter(full[i * BPC : (i + 1) * BPC], res.results[i]["out"])
    return full


def _install_ntff_hook_shim():
    """The image's antenv lacks axon_hooks; provide it so run_bass_kernel_spmd
    trace=True can capture NTFF profiles through the axon tunnel."""
    import sys as _sys
    import types

    try:
        from antenv.axon_hooks import get_axon_ntff_profile_hook  # noqa: F401

        return
    except ImportError:
        pass
    from trn_agent_boot.trn_boot import _ntff_profile_via_ctypes

    hook = _ntff_profile_via_ctypes("/opt/axon/libaxon_pjrt.so")
    mod = types.ModuleType("antenv.axon_hooks")
    mod.get_axon_ntff_profile_hook = lambda: hook
    mod.set_axon_ntff_profile_hook = lambda h: None
    _sys.modules["antenv.axon_hooks"] = mod


def _scatter_index():
    """Precompute (col, partition) -> flat output index maps per device col."""
    if "scatter" in _PROGRAM_CACHE:
        return _PROGRAM_CACHE["scatter"]
    rows = _row_map()
    all_cols = [(t, f) for t in range(NT) for f in range(F)]
    ncol = len(all_cols)
    b_of = np.zeros((ncol, 128), np.int64)
    feat_of = np.zeros((ncol, 128), np.int64)
    for c, (t, f) in enumerate(all_cols):
        for p in range(128):
            b, d_idx, k = rows[t][p]
            b_of[c, p] = b
            feat_of[c, p] = d_idx * K * F + k * F + f
    _PROGRAM_CACHE["scatter"] = (b_of, feat_of)
    return b_of, feat_of


def _scatter(dst, dev_out):
    """dev_out [128, NT*4] -> dst [BPC, 1344] in reference order."""
    dev = np.asarray(dev_out, np.float32)
    b_of, feat_of = _scatter_index()
    for c in range(feat_of.shape[0]):
        dst[b_of[:, :][c], feat_of[c]] = dev[:, c]


# revision 39
# speedup vs baseline: 1.2694x; 1.0724x over previous
"""MiniRocket feature extraction kernel for Trainium2 (8 NeuronCores, data parallel).

Contract: kernel(**inputs) takes the FULL inputs (as produced by setup_inputs())
and returns the FULL [64, 1344] float32 output. Internally the batch dim is
sharded 8-ways across the 8 NeuronCores; all other tensors are small replicated
constants that are preprocessed on the host into matmul weights / bias tables.

Math (per batch b, dilation d, kernel k, feature f):
    resp[k, l] = sum_{c,j} mask[d,k,c] * kern[k,j] * x[b, l + (j-4)*dil, c]
    feat[k, f] = w[k] * #{l in W_k : resp[k,l] > bias[d,k,f]}
    out        = (feat - mean) / std
where W_k is the full [0,L) window (even parity of d_idx+k, w=1/L) or the
interior [p, L-p) (odd parity, p = 4*dil, w=1/(L-2p)).

Device mapping (v4 -- 128-row packed PSUM, PSUM-direct counting, edge trick):
  - The per-(b,d) responses are packed 336-rows-per-batch into 24 logical
    PSUM tiles [128, 2048] (3 per batch) via quadrant-legal matmul sub-blocks
    (out partition offsets in {0,32,64,96}; <=32-row blocks anywhere, <=64-row
    blocks at {0,64}).  Counting cost is per-COLUMN, so 128-row tiles cut the
    count-op count from 128 to 96 vs the unpacked [84, *] layout.
  - The interior-window (odd-parity) trick is folded into the matmul: each
    patch carries a 73rd row holding the edge-indicator e_d[l] (1 on the
    2*pad edge columns), and the weight matrix gives that row -1e4 for
    odd-parity kernels.  Edge columns of odd rows come out of the matmul
    already poisoned below every bias -- no separate poison ops.
  - PPV counting reads resp DIRECTLY FROM PSUM, one op per (tile, feature):
      * DVE: tensor_scalar(is_gt, add, accum_out) -> direct count,
      * ACT: Sign(resp - b) with accum -> count = S/2 + L/2,
    with the (tile, feature) -> engine assignment chosen so both engines
    carry equal time (ACT ops are ~10% cheaper than DVE ops).
  - Final affine (count*A + B) folds the PPV weight, mean and std; A/B/bias
    tables are host-built per (tile-row, tile-feature-col) so dead partition
    rows (the 48 quadrant-packing crumbs per batch) are simply zeroed.
  - Patch tiles [73, 2048] rotate through 8 slots; slot s always serves
    dilation s%4, so row 72 (the e_d row) is written once upfront and the
    per-(b,d) patch DMA only rewrites rows 0..71 straight from DRAM.

walrus in this toolchain encodes at most ONE sync wait per compute/DMA
instruction; _legalize_sync_waits rewrites Tile's emitted waits to fit: a
transitive-closure (vector-clock) min-cover prunes redundant waits, extra
Matmult waits are hoisted onto the preceding Ldweights, and DMA waits park on
earlier free PE slots.  CRITICAL semantics baked into the pruner: an engine's
OWN semaphore tick is completion-level knowledge only and must never propagate
through the engine's instruction stream -- accumulator-drain aux ops (and
posted writes) lag the next instruction's dispatch on this silicon.
"""

import os
import sys

for _p in (
    "/root/.axon_site",
    "/root/.axon_site/_ro/trn_rl_repo",
    "/root/.axon_site/_ro/pypackages",
    "/opt/trn_rl_repo",
):
    if os.path.isdir(_p) and _p not in sys.path:
        sys.path.append(_p)

import numpy as np

B, L, C = 64, 2048, 8
DILATIONS = (1, 2, 4, 8)
D = 4
K = 84
F = 4
KERNEL_LEN = 9
NCORES = 8
BPC = B // NCORES  # batches per core
PAD = 32  # max shift = 4 * max(dil)
LP = L + 2 * PAD  # padded length
CE = C + 4  # channels + 4 edge-indicator pseudo-channels (one per dilation)
PR = CE * KERNEL_LEN  # patch rows (108): (c, j) pairs incl. edge rows
NT = (BPC * D * K) // 128  # 21 perfectly-packed [128, 2048] tiles per core
EDGE_W = -30000.0  # edge-poison weight on the edge rows (odd-parity kernels)

# Perfect packing: global row g = b*336 + d*84 + k maps to tile g//128,
# partition g%128 -- 2688 rows fill 21 tiles exactly.  Every matmul writes
# the FULL 128 partitions at offset 0 (always quadrant-legal) with zero
# weight columns outside its (b, d)-block's partition range; overlapping
# pieces within a tile compose via PSUM accumulation (start=False).


def _pieces():
    """List of (b, d, tile, row_a, row_b, k0): (b,d)-block parts per tile."""
    out = []
    for b in range(BPC):
        for d in range(D):
            g0 = b * (D * K) + d * K
            g1 = g0 + K
            g = g0
            while g < g1:
                t = g // 128
                ge = min(g1, (t + 1) * 128)
                out.append((b, d, t, g - 128 * t, ge - 128 * t, K - (g1 - g)))
                g = ge
    return sorted(out, key=lambda p: (p[2], p[0], p[1], p[3]))

_PROGRAM_CACHE: dict = {}


def _row_map():
    """(tile, partition) -> (b, d, k)."""
    m = [[None] * 128 for _ in range(NT)]
    for t in range(NT):
        for p in range(128):
            g = 128 * t + p
            b = g // (D * K)
            r = g % (D * K)
            m[t][p] = (b, r // K, r % K)
    return m


# Engine split: each [128, 1024] HALF-tile is read by exactly ONE engine
# (Tile serializes same-allocation reads from two engines -- a RAR artifact
# that would ping-pong DVE and ACT).  Half 0 of every tile -> DVE (is_gt
# counts), half 1 -> ACT (Sign counts); the two partial counts fold in the
# final affine: count = cnt0 + S1/2 + 512.


def _host_constants(kernels, channel_masks, bias_matrices, feature_mean, feature_std):
    """Build wT [73, D*K] f16 and cst [128, NT*4 * 4] f32 (bias, A_dve, A_act, B)."""
    kernels = np.asarray(kernels, np.float32)
    channel_masks = np.asarray(channel_masks, np.float32)
    bias_matrices = np.asarray(bias_matrices, np.float32)
    feature_mean = np.asarray(feature_mean, np.float32).reshape(D, K, F)
    feature_std = np.asarray(feature_std, np.float32).reshape(D, K, F)

    # weights: one [PR, 128] column-block per PIECE; within the block,
    # column p holds W[(c,j), k0+p-row_a] for p in [row_a, row_b) and zero
    # outside, so the matmul's full-128-partition write is a no-op on other
    # blocks' rows (they compose via PSUM accumulation).  Row 72+9*d (tap
    # j=0 of dilation d's edge pseudo-channel) = EDGE_W for odd-parity
    # kernels (their edge columns must count as "below every bias").
    pieces = _pieces()
    wfull = np.zeros((D, PR, K), np.float32)
    for d_idx in range(D):
        w = channel_masks[d_idx][:, :, None] * kernels[:, None, :]  # [K, C, 9]
        wfull[d_idx, 0:72, :] = w.reshape(K, C * KERNEL_LEN).T
        parity_odd = (d_idx + np.arange(K)) % 2 == 1
        wfull[d_idx, 72 + 9 * d_idx, :] = np.where(parity_odd, EDGE_W, 0.0)
    wT = np.zeros((PR, len(pieces) * 128), np.float16)
    for j, (b, d_idx, t, ra, rb, k0) in enumerate(pieces):
        wT[:, j * 128 + ra : j * 128 + rb] = wfull[d_idx][:, k0 : k0 + (rb - ra)].astype(
            np.float16
        )

    rows = _row_map()
    ncol = NT * F
    bias_d = np.zeros((128, ncol), np.float32)
    bias_a = np.zeros((128, ncol), np.float32)
    a_t = np.zeros((128, ncol), np.float32)
    a2_t = np.zeros((128, ncol), np.float32)
    b2_t = np.zeros((128, ncol), np.float32)
    for t in range(NT):
        for p in range(128):
            _b, d_idx, k = rows[t][p]
            pad = 4 * DILATIONS[d_idx]
            odd = (d_idx + k) % 2 == 1
            w_sel = 1.0 / (L - 2 * pad) if odd else 1.0 / L
            for f in range(F):
                i = t * F + f
                bb = bias_matrices[d_idx, k, f]
                mm = feature_mean[d_idx, k, f]
                ss = feature_std[d_idx, k, f]
                # count = cnt0 (DVE is_gt over half 0)
                #       + S1/2 + 512 (ACT Sign over half 1; edge poison
                #         contributes -1 like a below-bias sample)
                # out = (w*count - m)/s = cnt0*A + S1*A2 + B2
                bias_d[p, i] = bb
                bias_a[p, i] = -bb  # ACT bias is ADDED: Sign(resp + (-b))
                a_t[p, i] = w_sel / ss
                a2_t[p, i] = w_sel / (2.0 * ss)
                b2_t[p, i] = (w_sel * 512.0 - mm) / ss
    cst = np.concatenate([bias_d, bias_a, a_t, a2_t, b2_t], axis=1)
    return wT, cst


def _build_program():
    """Build the Bass/Tile program (same NEFF for all 8 cores)."""
    from contextlib import ExitStack

    import bass_rust
    import concourse.bass as bass
    import concourse.tile as tile
    from concourse import mybir

    f16 = mybir.dt.float16
    f32 = mybir.dt.float32
    A = mybir.AluOpType
    ACT = mybir.ActivationFunctionType

    ncol = NT * F

    nc = bass.Bass()
    xT = nc.declare_dram_parameter("xT", [BPC * CE, LP], f16, isOutput=False)
    wT = nc.declare_dram_parameter("wT", [PR, len(_pieces()) * 128], f16, isOutput=False)
    cst = nc.declare_dram_parameter("cst", [128, 5 * ncol], f32, isOutput=False)
    out = nc.declare_dram_parameter("out", [128, ncol], f32, isOutput=True)

    def patch_src(b, dil):
        """DRAM view: 9 dilation-shifted [CE, L] windows of batch b, c-major
        (includes the 4 edge pseudo-channels as rows 72..107)."""
        c = xT.ap().copy()
        c.offset = b * CE * LP + PAD - 4 * dil
        c.ap = bass_rust.VecI64Pair([[LP, CE], [dil, KERNEL_LEN], [1, L]])
        return c

    NSLOT = 8  # patch slots, reused every 2 batches via SWDGE

    with tile.TileContext(nc) as tc, ExitStack() as ctx:
        cpool = ctx.enter_context(tc.tile_pool(name="const", bufs=1))
        patch_pool = ctx.enter_context(tc.tile_pool(name="patch", bufs=1))
        psum_pool = ctx.enter_context(tc.tile_pool(name="psum", bufs=4, space="PSUM"))
        tr_pool = ctx.enter_context(tc.tile_pool(name="tr", bufs=8))
        tra_pool = ctx.enter_context(tc.tile_pool(name="tra", bufs=8))
        cnt_pool = ctx.enter_context(tc.tile_pool(name="cnt", bufs=1))
        osb_pool = ctx.enter_context(tc.tile_pool(name="osb", bufs=1))

        npieces = len(_pieces())
        nfirst = sum(1 for p in _pieces() if p[2] <= 2)
        wsb_a = cpool.tile([108, nfirst * 128], f16)
        wsb_b = cpool.tile([108, (npieces - nfirst) * 128], f16)
        wsrc = wT.ap()
        nc.sync.dma_start(wsb_a[:], wsrc[:, 0 : nfirst * 128])
        csb = cpool.tile([128, 5 * ncol], f32)

        # patch slots: patch bodies stream through the (otherwise idle)
        # GpSimd SWDGE queues IN PARALLEL with SP issuing the constants --
        # DMA issue costs ~600ns of engine time each, so splitting the
        # issue work across two engines shortens the lead-in.
        patches = [
            patch_pool.tile([108, L], f16, name=f"patch{s}") for s in range(NSLOT)
        ]

        def issue_patch(b, d_idx):
            s = (b * D + d_idx) % NSLOT
            nc.gpsimd.dma_start(patches[s][:], patch_src(b, DILATIONS[d_idx]))

        for b in range(2):
            for d_idx in range(D):
                issue_patch(b, d_idx)
        nc.sync.dma_start(wsb_b[:], wsrc[:, nfirst * 128 :])
        nc.sync.dma_start(csb[:], cst.ap())

        cnt_d = cnt_pool.tile([128, ncol], f32)
        cnt_a = cnt_pool.tile([128, ncol], f32)
        scr_d = cnt_pool.tile([128, 1], f32)
        scr_a = cnt_pool.tile([128, 1], f32)
        osb = osb_pool.tile([128, ncol], f32)

        # Touch csb once from DVE and ACT so its DMA-completion tick is in
        # both engines' vector clocks; later ops then carry at most one wait.
        nc.vector.tensor_copy(scr_d[:], csb[:, 0:1])
        nc.scalar.activation(scr_a[:], csb[0:128, 0:1], ACT.Copy)

        pieces = _pieces()
        by_tile = {}
        for j, pc in enumerate(pieces):
            by_tile.setdefault(pc[2], []).append((j, pc))
        # last piece index consuming each batch's patches (for prefetch)
        last_piece_of_batch = {}
        for j, (b, d_idx, t, ra, rb, k0) in enumerate(pieces):
            last_piece_of_batch[b] = max(last_piece_of_batch.get(b, -1), j)
        mm_boundaries = []
        mm_count = 0
        for t in range(NT):
            tp = by_tile[t]
            for h in range(2):
                ps = psum_pool.tile([128, 1024], f32, name="ps")
                for pi, (j, (b, d_idx, tt, ra, rb, k0)) in enumerate(tp):
                    patch = patches[(b * D + d_idx) % NSLOT]
                    for ch in (2 * h, 2 * h + 1):
                        cc = (ch - 2 * h) * 512
                        wtile, wj = (
                            (wsb_a, j) if j < nfirst else (wsb_b, j - nfirst)
                        )
                        nc.tensor.matmul(
                            ps[:, cc : cc + 512],
                            lhsT=wtile[:, wj * 128 : (wj + 1) * 128],
                            rhs=patch[:, ch * 512 : (ch + 1) * 512],
                            start=(pi == 0),
                            stop=(pi == len(tp) - 1),
                        )
                        mm_count += 1
                mm_boundaries.append(mm_count)
                # prefetch 2 batches out once batch b's patches are done with
                if h == 1:
                    for bb in range(BPC - 2):
                        if last_piece_of_batch[bb] in [j for j, _ in tp]:
                            for d_idx in range(D):
                                issue_patch(bb + 2, d_idx)
                for f in range(F):
                    i = t * F + f
                    if h == 0:
                        trash = tr_pool.tile([128, 1024], f16, name="trash")
                        nc.vector.tensor_scalar(
                            trash[:],
                            ps[:],
                            csb[:, i : i + 1],
                            None,
                            A.is_gt,
                            A.add,
                            accum_out=cnt_d[:, i : i + 1],
                        )
                    else:
                        trash_a = tra_pool.tile([128, 1024], f16, name="trash_a")
                        nc.scalar.activation(
                            trash_a[:],
                            ps[:],
                            ACT.Sign,
                            bias=csb[:, ncol + i : ncol + i + 1],
                            accum_out=cnt_a[:, i : i + 1],
                        )
        nc._mm_boundaries = set(mm_boundaries)

        # fold + affine: osb = cnt0*A + S1*A2 + B2  (count = cnt0 + S1/2
        # + 512 folded into the tables; host unscrambles columns)
        nc.vector.tensor_tensor(
            cnt_d[:], cnt_d[:], csb[:, 2 * ncol : 3 * ncol], A.mult
        )
        nc.vector.tensor_tensor(
            cnt_a[:], cnt_a[:], csb[:, 3 * ncol : 4 * ncol], A.mult
        )
        nc.vector.tensor_tensor(osb[:], cnt_d[:], cnt_a[:], A.add)
        nc.vector.tensor_tensor(
            osb[:], osb[:], csb[:, 4 * ncol : 5 * ncol], A.add
        )

        nc.sync.dma_start(out.ap(), osb[:])

    _legalize_sync_waits(nc, bass_rust)
    return nc


def _legalize_sync_waits(nc, bass_rust):
    """walrus encodes at most ONE sync wait per compute/DMA instruction.
    Rewrites, validated in the CoreSim race detector and on hardware:
     1. Transitive-closure (vector-clock) min-cover prunes redundant waits.
     2. Extra Matmult waits hoist onto the immediately-preceding Ldweights.
     3. Remaining multi-waits on DMAs park on earlier free PE slots.
     4. Kernel-tail SP drain waits prune to (at most) the output-store queue.
    """
    blocks = list(nc.m.functions[0].blocks)
    end_blk = next(b for b in blocks if b.name.endswith("_end"))

    max_waited: dict = {}
    for blk in blocks:
        if blk is end_blk:
            continue
        for inst in blk.instructions:
            si = inst.sync_info
            for w in si.on_wait if si and si.on_wait else []:
                if w.wait_value > max_waited.get(w.ant_name, -1):
                    max_waited[w.ant_name] = w.wait_value

    body = [b for b in blocks if b is not end_blk and not b.name == "main"]
    know_after: dict = {}  # stream knowledge (excludes own sem: accum aux lag)
    know_full: dict = {}  # completion knowledge (includes own sem updates)
    producers: dict = {}  # sem -> list of (value, inst_idx, is_dma)
    prev_on_engine: dict = {}
    eng_stream: dict = {}  # engine -> its instructions in program order
    insts = [i for b in body for i in b.instructions]

    def covered(know, sem, val):
        return know.get(sem, -1) >= val

    for idx, inst in enumerate(insts):
        eng = str(inst.engine).split(".")[-1]
        si = inst.sync_info
        is_dma = inst.opcode == "DMACopy"
        know = dict(know_after.get(prev_on_engine.get(eng), {}))
        waits = list(si.on_wait) if si and si.on_wait else []
        if waits:
            # knowledge each wait would contribute
            contrib = []
            for w in waits:
                c = {}
                for v, pidx, pdma in producers.get(w.ant_name, []):
                    if v >= w.wait_value:
                        c = dict(know_full.get(pidx, {}))
                        break
                c[w.ant_name] = max(c.get(w.ant_name, -1), w.wait_value)
                contrib.append(c)
            # smallest subset of waits whose merged transitive knowledge
            # (plus same-engine knowledge) covers every wait
            from itertools import combinations

            need = [
                i
                for i, w in enumerate(waits)
                if not covered(know, w.ant_name, w.wait_value)
            ]
            best = None
            for sz in range(0, len(need) + 1):
                for sub in combinations(need, sz):
                    merged = dict(know)
                    for i in sub:
                        for s, v in contrib[i].items():
                            if merged.get(s, -1) < v:
                                merged[s] = v
                    if all(
                        covered(merged, waits[i].ant_name, waits[i].wait_value)
                        for i in need
                    ):
                        best = (sub, merged)
                        break
                if best is not None:
                    break
            assert best is not None
            know = best[1]
            waits = [waits[i] for i in best[0]]
        if len(waits) > 1:
            # Hoist extra waits onto earlier wait-free instructions of the
            # SAME engine (engines execute in order, so a wait satisfied
            # before an earlier instruction is satisfied before this one).
            # DMA-queue waits may park anywhere AFTER the producing enqueue
            # (the enqueue does not depend on this engine, so no cycle);
            # engine-sem waits keep a tight 8-instruction window, inside
            # which wait producers depend only on work preceding the window.
            eng_insts = eng_stream.get(eng, [])
            waits.sort(key=lambda w: not w.ant_name.startswith("DMA"))
            kept_w = []
            while len(kept_w) + len(waits) > 1 and waits:
                w = waits.pop(0)
                if w.ant_name.startswith("DMA"):
                    plist = producers.get(w.ant_name, [])
                    pidx = -1
                    for v, pi, pdma in plist:
                        if v >= w.wait_value:
                            pidx = pi
                            break
                    lo = 0
                    while lo < len(eng_insts) and eng_insts[lo][0] <= pidx:
                        lo += 1
                    lo = max(lo, 0)
                else:
                    lo = max(0, len(eng_insts) - 8)
                placed = False
                for j in range(len(eng_insts) - 1, lo - 1, -1):
                    cand = eng_insts[j][1]
                    csi = cand.sync_info
                    if csi is not None and csi.on_wait:
                        continue
                    if csi is None:
                        csi = bass_rust.SyncInfo(on_wait=[], on_update=[])
                        cand.sync_info = csi
                    csi.on_wait = [w]
                    placed = True
                    break
                if not placed:
                    if os.environ.get("LEGALIZE_DEBUG"):
                        print(
                            f"DEBUG place-fail {inst.name} w={w.ant_name}>={w.wait_value} "
                            f"lo={lo} n_eng={len(eng_insts)} "
                            f"tail_busy={[(g, i.opcode, bool(i.sync_info and i.sync_info.on_wait)) for g, i in eng_insts[max(0,lo):][-12:]]}"
                        )
                    kept_w.append(w)
            waits = kept_w + waits
        assert len(waits) <= 1, (
            f"{inst.name} {inst.opcode} still has waits "
            f"{[(w.ant_name, w.wait_value) for w in waits]}"
        )
        if si is not None:
            si.on_wait = waits
        elif waits:
            inst.sync_info = bass_rust.SyncInfo(on_wait=waits, on_update=[])
        # record updates (update_value is an INCREMENT; waits are cumulative
        # thresholds, so track running totals per semaphore). An instruction
        # with an accumulator output drains it via a lagging aux op: its sem
        # tick is completion-level knowledge only and must NOT propagate
        # through the engine stream (the next instruction may start first).
        # DMA enqueues complete asynchronously.
        full = dict(know)
        if si and si.on_update:
            for u in si.on_update:
                plist = producers.setdefault(u.ant_name, [])
                total = (plist[-1][0] if plist else 0) + u.update_value
                plist.append((total, idx, is_dma))
                if not is_dma:
                    if full.get(u.ant_name, -1) < total:
                        full[u.ant_name] = total
        know_after[idx] = know
        know_full[idx] = full
        prev_on_engine[eng] = idx
        eng_stream.setdefault(eng, []).append((idx, inst))

    _thin_pe_stream(nc, insts, blocks, end_blk)

    # (4) tail drain
    end_insts = list(end_blk.instructions)
    tail = end_insts[0]
    assert tail.opcode == "Drain", f"unexpected end block head {tail.opcode}"
    si = tail.sync_info
    if si and len(si.on_wait) > 1:
        eng_pfx = ("Activation_", "PE_", "DVE_", "Pool_", "SP_")
        keep = [
            w
            for w in si.on_wait
            if not w.ant_name.startswith(eng_pfx)
            and max_waited.get(w.ant_name, -1) < w.wait_value
        ]
        if len(keep) > 1:
            # spill extras onto zero-wait drains before the sem reset
            spill_slots = []
            for inst in end_insts[1:]:
                if inst.opcode == "ISA":
                    break
                isi = inst.sync_info
                if inst.opcode == "Drain" and (not isi or not isi.on_wait):
                    spill_slots.append(inst)
            assert len(spill_slots) >= len(keep) - 1, (
                f"tail drain needs {len(keep)} wait slots, "
                f"only {1 + len(spill_slots)} available"
            )
            for w, slot in zip(keep[1:], spill_slots):
                ssi = slot.sync_info
                if ssi is None:
                    ssi = bass_rust.SyncInfo(on_wait=[], on_update=[])
                    slot.sync_info = ssi
                ssi.on_wait = [w]
            keep = keep[:1]
        si.on_wait = keep


def _thin_pe_stream(nc, insts, blocks, end_blk):
    """Cut PE-stream overhead, per the tensor-engine guidance that
    per-matmul semaphore increments serialize (~26ns each) and break
    back-to-back matmul pipelining:
      1. Keep the PE completion-sem update only on each PSUM tile's LAST
         matmul (matmuls complete in pc order, so the last tick implies the
         rest); remap every wait on that sem accordingly (rounding a wait up
         to its tile's last matmul is always sound -- it waits longer).
      2. Drop a Ldweights when the previous surviving PE instruction chain
         loads the SAME weights AP (consecutive matmuls reuse the loaded
         weights); only parameter-free ones (no waits, no updates) go.
    """
    boundaries = getattr(nc, "_mm_boundaries", set())
    # find the PE completion sem: the one Matmults update
    pe_sem = None
    for inst in insts:
        if inst.opcode == "Matmult" and inst.sync_info and inst.sync_info.on_update:
            pe_sem = inst.sync_info.on_update[0].ant_name
            break
    if pe_sem is None:
        return
    # pass 1: strip non-tile-last matmul updates, build old->new total map
    old_total = 0
    kept_totals = []  # (old_total_reached, new_total)
    new_total = 0
    for inst in insts:
        si = inst.sync_info
        ups = list(si.on_update) if si and si.on_update else []
        for u in ups:
            if u.ant_name != pe_sem:
                continue
            old_total += u.update_value
            if inst.opcode == "Matmult" and old_total not in boundaries:
                si.on_update = [x for x in si.on_update if x is not u]
            else:
                new_total += u.update_value
                kept_totals.append((old_total, new_total))

    def remap(v):
        for old, new in kept_totals:
            if old >= v:
                return new
        return kept_totals[-1][1] if kept_totals else v

    for blk in blocks:
        for inst in blk.instructions:
            si = inst.sync_info
            if not si or not si.on_wait:
                continue
            changed = False
            ws = []
            for w in si.on_wait:
                if w.ant_name == pe_sem:
                    nv = remap(w.wait_value)
                    if nv != w.wait_value:
                        w.wait_value = nv
                        changed = True
                ws.append(w)
            if changed:
                si.on_wait = ws

    # pass 2: dedup consecutive identical Ldweights on the PE stream
    pe_insts = [
        i for i in insts if str(i.engine).split(".")[-1] == "PE"
    ]
    last_w = None
    to_remove = set()
    for inst in pe_insts:
        if inst.opcode == "Ldweights":
            si = inst.sync_info
            key = str(inst.ins[0])
            if (
                key == last_w
                and not (si and (si.on_wait or si.on_update))
            ):
                to_remove.add(id(inst))
            last_w = key
        elif inst.opcode != "Matmult":
            last_w = None  # barriers etc. may clobber the PE array state
    if to_remove:
        for blk in blocks:
            kept = [i for i in blk.instructions if id(i) not in to_remove]
            if len(kept) != len(list(blk.instructions)):
                blk.instructions = kept


def _get_program():
    if "nc" not in _PROGRAM_CACHE:
        _PROGRAM_CACHE["nc"] = _build_program()
    return _PROGRAM_CACHE["nc"]


def _prep_x(x):
    """[64, 2048, 8] f32 -> per-core [BPC*12, 2112] f16 slices.

    Channels 0..7 are the (padded, channel-major) data; channels 8..11 hold
    the per-dilation edge-indicator patterns U_d with U_d[PAD-4*dil+l] =
    e_d[l], so the patch AP picks up each dilation's e-row as tap j=0 of its
    fake channel -- one DMA covers data AND edge rows.
    """
    xt = np.ascontiguousarray(np.asarray(x, np.float32).transpose(0, 2, 1))
    xp = np.zeros((B, CE, LP), np.float16)
    xp[:, :C, PAD : PAD + L] = xt.astype(np.float16)
    for d_idx, dil in enumerate(DILATIONS):
        pad = 4 * dil
        e = np.zeros(L, np.float16)
        e[:pad] = 1.0
        e[L - pad :] = 1.0
        xp[:, C + d_idx, PAD - pad : PAD - pad + L] = e
    return [
        xp[i * BPC : (i + 1) * BPC].reshape(BPC * CE, LP) for i in range(NCORES)
    ]


def kernel(
    x,
    kernels,
    channel_masks,
    bias_matrices,
    feature_mean,
    feature_std,
    _trace=False,
    _sim=False,
):
    wT, cst = _host_constants(
        kernels, channel_masks, bias_matrices, feature_mean, feature_std
    )
    x_slices = _prep_x(x)
    nc = _get_program()

    in_maps = [
        {"xT": x_slices[i], "wT": wT, "cst": cst}
        for i in range(NCORES)
    ]

    if _sim:
        import concourse.bass_interp as bass_interp

        try:
            nc.detect_race_conditions = False
        except Exception:
            pass
        sim = bass_interp.MultiCoreSim(nc, 1)
        sim.cores[0].assign_tensors(in_maps[0])
        sim.simulate()
        dev_outs = [np.array(sim.cores[0].tensor("out"))]
        full = np.zeros((B, 1344), np.float32)
        _scatter(full[:BPC], dev_outs[0])
        _PROGRAM_CACHE["exec_time_ns"] = None
        return full

    if _trace:
        _install_ntff_hook_shim()

    from concourse.bass_utils import run_bass_kernel_spmd

    res = run_bass_kernel_spmd(
        nc,
        in_maps,
        core_ids=list(range(NCORES)),
        trace=_trace,
        trace_cores=list(range(NCORES)) if _trace else None,
    )
    _PROGRAM_CACHE["exec_time_ns"] = res.exec_time_ns
    _PROGRAM_CACHE["mean_exec_time_ns"] = res.mean_exec_time_ns
    _PROGRAM_CACHE["trace"] = res.instructions_and_trace

    full = np.empty((B, 1344), np.float32)
    for i in range(NCORES):
        _scatter(full[i * BPC : (i + 1) * BPC], res.results[i]["out"])
    return full


def _install_ntff_hook_shim():
    """The image's antenv lacks axon_hooks; provide it so run_bass_kernel_spmd
    trace=True can capture NTFF profiles through the axon tunnel."""
    import sys as _sys
    import types

    try:
        from antenv.axon_hooks import get_axon_ntff_profile_hook  # noqa: F401

        return
    except ImportError:
        pass
    from trn_agent_boot.trn_boot import _ntff_profile_via_ctypes

    hook = _ntff_profile_via_ctypes("/opt/axon/libaxon_pjrt.so")
    mod = types.ModuleType("antenv.axon_hooks")
    mod.get_axon_ntff_profile_hook = lambda: hook
    mod.set_axon_ntff_profile_hook = lambda h: None
    _sys.modules["antenv.axon_hooks"] = mod


def _scatter_index():
    """Precompute (col, partition) -> flat output index maps per device col."""
    if "scatter" in _PROGRAM_CACHE:
        return _PROGRAM_CACHE["scatter"]
    rows = _row_map()
    all_cols = [(t, f) for t in range(NT) for f in range(F)]
    ncol = len(all_cols)
    b_of = np.zeros((ncol, 128), np.int64)
    feat_of = np.zeros((ncol, 128), np.int64)
    for c, (t, f) in enumerate(all_cols):
        for p in range(128):
            b, d_idx, k = rows[t][p]
            b_of[c, p] = b
            feat_of[c, p] = d_idx * K * F + k * F + f
    _PROGRAM_CACHE["scatter"] = (b_of, feat_of)
    return b_of, feat_of


def _scatter(dst, dev_out):
    """dev_out [128, NT*4] -> dst [BPC, 1344] in reference order."""
    dev = np.asarray(dev_out, np.float32)
    b_of, feat_of = _scatter_index()
    for c in range(feat_of.shape[0]):
        dst[b_of[:, :][c], feat_of[c]] = dev[:, c]
